# revision 8
# baseline (speedup 1.0000x reference)
"""Trainium2 Bass kernel for nn_AdaptiveKernelModule (dense_cnn).

Math: the per-sample dynamic conv kernel is rank-2 in its output channel:
    gk[o,i,kh,kw] = Wk[o] * g[i,kh,kw] + bk[o]
so with u = Wf@Wk, v = Wf@bk, w = Wf@b_adap + bf (host-precomputed):
    out[c, p] = u[c] * A[p] + v[c] * B[p] + w[c]
    A[p] = sum_{i,kh,kw} g[i,kh,kw] * f[i, p + delta(kh,kw)]
    B[p] = sum_{i,kh,kw}              f[i, p + delta(kh,kw)]
    f    = relu(W1 @ x + b1)

Device pipeline per sample (2 samples per core, 8 cores data-parallel over N):
  DMA x tile in (f32) -> GPSIMD cast to bf16
  MM1: f_psum = W1Tx4.T @ xb_chunk   (bf16, K=128, M=128 replicated x4),
       ACT relu+b1 on the c0 block -> f_pad bf16 (SBUF, zero border)
  maxpool 64x64 windows on DVE (bf16) -> xp[128,10]; tiny MM + relu -> g
  MM2: T_psum = G2.T @ f_pad_chunk  (bf16, K=32, M=128; cols 96..104 = g taps,
       col 105 = ones, rest zero), DVE copy rows 96..105 -> T_plain bf16
  DMA SBUF->SBUF: T_sb[t, q] = T_plain[row(t), q + delta_t]  (18 shifted rows)
  MM3: out_psum = L3.T @ T_sb_chunk  (bf16, K=18, M=128), L3 = [u]*9 + [v]*9,
       ACT Identity+bias(w) -> f32 out tile -> DMA to HBM.
bf16 matmuls run 1 cyc/row at 2.4 GHz with fast weight load; fp32 paths on
this chip run 2 cyc/row at 1.2 GHz (fp32r) or 4 cyc/row, so bf16 is ~4x.
"""

import numpy as np
import ml_dtypes

import concourse.bass as bass
import concourse.bacc as bacc
import concourse.mybir as mybir
import concourse.tile as tile
from concourse.bass_utils import run_bass_kernel_spmd

F32 = mybir.dt.float32
BF16 = mybir.dt.bfloat16

N_CORES = 8
NS = 2            # samples per core
C = 128           # input channels
CM = 32           # bottleneck channels
H = W = 192
HP = WP = 194     # padded
L = HP * WP       # padded pixels per plane (37636)
XROWS = 16        # image rows per x/out tile
RROWS = 2         # image rows per matmul chunk (N = 2*192 = 384)

# partition layout inside the mega (bf16) tile
FP0, FP1 = 0, 32      # f_pad for sample buffer 0 / 1 (32 partitions each)
TSB = 64              # T_sb: 18 shifted tap rows
TPL = 96              # T_plain: 10 raw rows (9 g-taps + 1 ones-tap)
GCOL = 96             # G2 tap columns 96..104, ones col 105

DELTAS = [(kh - 1) * WP + (kw - 1) for kh in range(3) for kw in range(3)]


def build(nc):
    x_d = nc.declare_dram_parameter("x", [NS, C, H, W], F32, isOutput=False)
    w1t_d = nc.declare_dram_parameter("w1t", [C, C], BF16, isOutput=False)
    b1_d = nc.declare_dram_parameter("b1", [CM, 1], F32, isOutput=False)
    l3_d = nc.declare_dram_parameter("l3", [18, C], BF16, isOutput=False)
    wb_d = nc.declare_dram_parameter("wb", [C, 1], F32, isOutput=False)
    out_d = nc.declare_dram_parameter("out", [NS, C, H, W], F32, isOutput=True)

    with tile.TileContext(nc) as tc:
        with (
            tc.tile_pool(name="persist", bufs=1) as pp,
            tc.tile_pool(name="xin", bufs=3) as xin_pool,
            tc.tile_pool(name="xbf", bufs=3) as xbf_pool,
            tc.tile_pool(name="outp", bufs=2) as out_pool,
            tc.tile_pool(name="small", bufs=2) as sp,
            tc.tile_pool(name="psum", bufs=2, space="PSUM") as psp,
        ):
            mega = pp.tile([128, L], BF16)
            w1t_sb = pp.tile([C, C], BF16)
            b1_sb = pp.tile([64, 1], F32)
            l3_sb = pp.tile([128, C], BF16)
            wb_sb = pp.tile([C, 1], F32)
            g2 = pp.tile([64, C], BF16)

            nc.sync.dma_start(out=w1t_sb[:, :], in_=w1t_d.ap())
            nc.sync.dma_start(out=b1_sb[0:32, :], in_=b1_d.ap())
            nc.sync.dma_start(out=b1_sb[32:64, :], in_=b1_d.ap())
            nc.sync.dma_start(out=l3_sb[64:82, :], in_=l3_d.ap())
            nc.sync.dma_start(out=wb_sb[:, :], in_=wb_d.ap())

            # G2 stationary operand: zero everywhere, ones in col GCOL+9;
            # per-sample g taps land in cols GCOL..GCOL+8 via the ACT evac.
            nc.vector.memset(g2[:, :], 0.0)
            nc.vector.memset(g2[:, GCOL + 9 : GCOL + 10], 1.0)

            # zero the f_pad borders for both sample buffers (never overwritten)
            meg3 = mega[0:64, :].rearrange("p (r c) -> p r c", c=WP)
            nc.vector.memset(mega[0:64, 0:WP], 0.0)
            nc.vector.memset(mega[0:64, (HP - 1) * WP : HP * WP], 0.0)
            nc.vector.memset(meg3[:, :, 0:1], 0.0)
            nc.vector.memset(meg3[:, :, WP - 1 : WP], 0.0)

            for n in range(NS):
                c0 = FP0 if n % 2 == 0 else FP1
                fpad = mega[c0 : c0 + CM, :]
                fpad3 = fpad.rearrange("p (r c) -> p r c", c=WP)
                b1n = b1_sb[c0 : c0 + CM, :]

                # ---------------- pass 1: x in, cast, maxpool partials, MM1+relu
                ntiles = H // XROWS  # 12
                xp_part = sp.tile([128, 3 * ntiles], F32, tag="xp_part")
                for j in range(ntiles):
                    xt = xin_pool.tile([128, XROWS * W], F32, tag="xt")
                    xt3 = xt.rearrange("p (r c) -> p r c", c=W)
                    nc.sync.dma_start(
                        out=xt3, in_=x_d.ap()[n, :, j * XROWS : (j + 1) * XROWS, :]
                    )
                    xb = xbf_pool.tile([128, XROWS * W], BF16, tag="xb")
                    nc.vector.tensor_copy(xb[:, :], xt[:, :])
                    # maxpool partial over this 16-row band: out [128, 3]
                    xb4 = xb.rearrange("p (r kx c) -> p kx r c", kx=3, c=64)
                    nc.vector.tensor_reduce(
                        xp_part[:, 3 * j : 3 * j + 3],
                        xb4,
                        axis=mybir.AxisListType.XY,
                        op=mybir.AluOpType.max,
                    )
                    for r in range(XROWS // RROWS):
                        y0 = j * XROWS + r * RROWS
                        pf = psp.tile([128, 512], F32, tag="pf", name="pf")[
                            :, : RROWS * W
                        ]
                        nc.tensor.matmul(
                            pf[:, :],
                            w1t_sb[:, :],
                            xb[:, r * RROWS * W : (r + 1) * RROWS * W],
                        )
                        nc.scalar.activation(
                            fpad3[:, y0 + 1 : y0 + 1 + RROWS, 1 : 1 + W],
                            pf[c0 : c0 + CM, :].rearrange("p (r c) -> p r c", c=W),
                            mybir.ActivationFunctionType.Relu,
                            bias=b1n,
                        )

                # ---------------- finalize maxpool, compute g
                xp_r = sp.tile([128, 10], BF16, tag="xp_r")
                nc.vector.memset(xp_r[:, 9:10], 0.0)
                nc.vector.tensor_reduce(
                    xp_r[:, 0:9],
                    xp_part.rearrange(
                        "p (ky s kx) -> p ky kx s", ky=3, kx=3
                    ),
                    axis=mybir.AxisListType.X,
                    op=mybir.AluOpType.max,
                )
                pg = psp.tile([128, 512], F32, tag="pg", name="pg")[:, :10]
                nc.tensor.matmul(pg[:, :], w1t_sb[:, :], xp_r[:, :])
                nc.scalar.activation(
                    g2[c0 : c0 + CM, GCOL : GCOL + 9],
                    pg[c0 : c0 + CM, 0:9],
                    mybir.ActivationFunctionType.Relu,
                    bias=b1n,
                )

                # ---------------- MM2: T = G2.T @ f_pad, all padded rows
                tpl = mega[TPL : TPL + 10, :]
                for p0 in range(0, HP, RROWS):
                    pT = psp.tile([128, 512], F32, tag="pT", name="pT")[
                        :, : RROWS * WP
                    ]
                    nc.tensor.matmul(
                        pT[:, :],
                        g2[c0 : c0 + CM, :],
                        fpad[:, p0 * WP : (p0 + RROWS) * WP],
                    )
                    nc.vector.tensor_copy(
                        tpl[:, p0 * WP : (p0 + RROWS) * WP],
                        pT[TPL : TPL + 10, :],
                    )

                # ---------------- shifted tap copies SBUF->SBUF (HWDGE on sync)
                for t in range(18):
                    src = TPL + (t if t < 9 else 9)
                    d = DELTAS[t % 9]
                    a = max(0, -d)
                    b = L - max(0, d)
                    nc.sync.dma_start(
                        out=mega[TSB + t : TSB + t + 1, a:b],
                        in_=mega[src : src + 1, a + d : b + d],
                    )

                # ---------------- MM3 + bias + store
                tsb = mega[TSB : TSB + 18, :].rearrange("p (r c) -> p r c", c=WP)
                for j in range(ntiles):
                    ot = out_pool.tile([128, XROWS * W], F32, tag="ot")
                    for r in range(XROWS // RROWS):
                        y0 = j * XROWS + r * RROWS
                        po = psp.tile([128, 512], F32, tag="po", name="po")[
                            :, : RROWS * W
                        ]
                        nc.tensor.matmul(
                            po[:, :],
                            l3_sb[64:82, :],
                            tsb[:, y0 + 1 : y0 + 1 + RROWS, 1 : 1 + W],
                        )
                        nc.scalar.activation(
                            ot[:, r * RROWS * W : (r + 1) * RROWS * W],
                            po[:, :],
                            mybir.ActivationFunctionType.Identity,
                            bias=wb_sb[:, :],
                        )
                    nc.sync.dma_start(
                        out=out_d.ap()[n, :, j * XROWS : (j + 1) * XROWS, :],
                        in_=ot.rearrange("p (r c) -> p r c", c=W),
                    )
    return nc


_CACHE = {}


def _get_nc():
    if "nc" not in _CACHE:
        nc = bacc.Bacc(
            "TRN2", target_bir_lowering=False, debug=False, num_devices=N_CORES
        )
        build(nc)
        nc.compile()
        _CACHE["nc"] = nc
    return _CACHE["nc"]


def make_in_maps(x, W1, b1, Wk, bk, b_adap, Wf, bf):
    x = np.asarray(x, dtype=np.float32)
    W1 = np.asarray(W1, dtype=np.float32)
    b1 = np.asarray(b1, dtype=np.float32)
    Wk = np.asarray(Wk, dtype=np.float32)
    bk = np.asarray(bk, dtype=np.float32)
    b_adap = np.asarray(b_adap, dtype=np.float32)
    Wf = np.asarray(Wf, dtype=np.float32)
    bf = np.asarray(bf, dtype=np.float32)

    u = Wf @ Wk                # [128]
    v = Wf @ bk                # [128]
    w = Wf @ b_adap + bf       # [128]
    l3 = np.ascontiguousarray(
        np.stack([u] * 9 + [v] * 9).astype(ml_dtypes.bfloat16)
    )
    w1t = np.ascontiguousarray(
        np.tile(W1.T, (1, 4)).astype(ml_dtypes.bfloat16)
    )
    b1c = np.ascontiguousarray(b1[:, None].astype(np.float32))
    wbc = np.ascontiguousarray(w[:, None].astype(np.float32))

    in_maps = []
    for i in range(N_CORES):
        in_maps.append(
            {
                "x": np.ascontiguousarray(x[i * NS : (i + 1) * NS]),
                "w1t": w1t,
                "b1": b1c,
                "l3": l3,
                "wb": wbc,
            }
        )
    return in_maps


def kernel(x, W1, b1, Wk, bk, b_adap, Wf, bf):
    nc = _get_nc()
    in_maps = make_in_maps(x, W1, b1, Wk, bk, b_adap, Wf, bf)
    res = run_bass_kernel_spmd(nc, in_maps, list(range(N_CORES)))
    return np.concatenate([res.results[i]["out"] for i in range(N_CORES)], axis=0)


# revision 9
# speedup vs baseline: 8.0229x; 8.0229x over previous
"""Trainium2 Bass kernel for nn_AdaptiveKernelModule (dense_cnn).

Math: the per-sample dynamic conv kernel is rank-2 in its output channel:
    gk[o,i,kh,kw] = Wk[o] * g[i,kh,kw] + bk[o]
so with u = Wf@Wk, v = Wf@bk, w = Wf@b_adap + bf (host-precomputed):
    out[c, p] = u[c] * A[p] + v[c] * B[p] + w[c]
    A[p] = sum_{i,kh,kw} g[i,kh,kw] * f[i, p + delta(kh,kw)]
    B[p] = sum_{i,kh,kw}              f[i, p + delta(kh,kw)]
    f    = relu(W1 @ x + b1)

Device pipeline per sample (2 samples per core, 8 cores data-parallel over N):
  DMA x tile in (f32) -> GPSIMD cast to bf16
  MM1: f_psum = W1Tx4.T @ xb_chunk   (bf16, K=128, M=128 replicated x4),
       ACT relu+b1 on the c0 block -> f_pad bf16 (SBUF, zero border)
  maxpool 64x64 windows on DVE (bf16) -> xp[128,10]; tiny MM + relu -> g
  MM2: T_psum = G2.T @ f_pad_chunk  (bf16, K=32, M=128; cols 96..104 = g taps,
       col 105 = ones, rest zero), DVE copy rows 96..105 -> T_plain bf16
  DMA SBUF->SBUF: T_sb[t, q] = T_plain[row(t), q + delta_t]  (18 shifted rows)
  MM3: out_psum = L3.T @ T_sb_chunk  (bf16, K=18, M=128), L3 = [u]*9 + [v]*9,
       ACT Identity+bias(w) -> f32 out tile -> DMA to HBM.
bf16 matmuls run 1 cyc/row at 2.4 GHz with fast weight load; fp32 paths on
this chip run 2 cyc/row at 1.2 GHz (fp32r) or 4 cyc/row, so bf16 is ~4x.
"""

import numpy as np
import ml_dtypes

import concourse.bass as bass
import concourse.bacc as bacc
import concourse.mybir as mybir
import concourse.tile as tile
from concourse.bass_utils import run_bass_kernel_spmd

F32 = mybir.dt.float32
BF16 = mybir.dt.bfloat16

N_CORES = 8
NS = 2            # samples per core
C = 128           # input channels
CM = 32           # bottleneck channels
H = W = 192
HP = WP = 194     # padded
L = HP * WP       # padded pixels per plane (37636)
XROWS = 16        # image rows per x/out tile
RROWS = 2         # image rows per matmul chunk (N = 2*192 = 384)

# partition layout inside the mega (bf16) tile
FP0, FP1 = 0, 32      # f_pad for sample buffer 0 / 1 (32 partitions each)
TSB = 64              # T_sb: 18 shifted tap rows
TPL = 96              # T_plain: 10 raw rows (9 g-taps + 1 ones-tap)
GCOL = 96             # G2 tap columns 96..104, ones col 105

DELTAS = [(kh - 1) * WP + (kw - 1) for kh in range(3) for kw in range(3)]


def build(nc):
    x_d = nc.declare_dram_parameter("x", [NS, C, H, W], F32, isOutput=False)
    w1t_d = nc.declare_dram_parameter("w1t", [C, C], BF16, isOutput=False)
    b1_d = nc.declare_dram_parameter("b1", [CM, 1], F32, isOutput=False)
    l3_d = nc.declare_dram_parameter("l3", [18, C], BF16, isOutput=False)
    wb_d = nc.declare_dram_parameter("wb", [C, 1], F32, isOutput=False)
    out_d = nc.declare_dram_parameter("out", [NS, C, H, W], F32, isOutput=True)

    with tile.TileContext(nc) as tc:
        with (
            tc.tile_pool(name="persist", bufs=1) as pp,
            tc.tile_pool(name="xin", bufs=3) as xin_pool,
            tc.tile_pool(name="xbf", bufs=3) as xbf_pool,
            tc.tile_pool(name="outp", bufs=2) as out_pool,
            tc.tile_pool(name="small", bufs=2) as sp,
            tc.tile_pool(name="psum", bufs=2, space="PSUM") as psp,
        ):
            mega = pp.tile([128, L], BF16)
            w1t_sb = pp.tile([C, C], BF16)
            b1_sb = pp.tile([64, 1], F32)
            l3_sb = pp.tile([128, C], BF16)
            wb_sb = pp.tile([C, 1], F32)
            g2 = pp.tile([64, C], BF16)

            nc.sync.dma_start(out=w1t_sb[:, :], in_=w1t_d.ap())
            nc.sync.dma_start(out=b1_sb[0:32, :], in_=b1_d.ap())
            nc.sync.dma_start(out=b1_sb[32:64, :], in_=b1_d.ap())
            nc.sync.dma_start(out=l3_sb[64:82, :], in_=l3_d.ap())
            nc.sync.dma_start(out=wb_sb[:, :], in_=wb_d.ap())

            # G2 stationary operand: zero everywhere, ones in col GCOL+9;
            # per-sample g taps land in cols GCOL..GCOL+8 via the ACT evac.
            nc.vector.memset(g2[:, :], 0.0)
            nc.vector.memset(g2[:, GCOL + 9 : GCOL + 10], 1.0)

            # zero the f_pad borders for both sample buffers (never overwritten)
            meg3 = mega[0:64, :].rearrange("p (r c) -> p r c", c=WP)
            nc.vector.memset(mega[0:64, 0:WP], 0.0)
            nc.vector.memset(mega[0:64, (HP - 1) * WP : HP * WP], 0.0)
            nc.vector.memset(meg3[:, :, 0:1], 0.0)
            nc.vector.memset(meg3[:, :, WP - 1 : WP], 0.0)

            for n in range(NS):
                c0 = FP0 if n % 2 == 0 else FP1
                fpad = mega[c0 : c0 + CM, :]
                fpad3 = fpad.rearrange("p (r c) -> p r c", c=WP)
                b1n = b1_sb[c0 : c0 + CM, :]

                # ---------------- pass 1: x in, cast, maxpool partials, MM1+relu
                ntiles = H // XROWS  # 12
                xp_part = sp.tile([128, 3 * ntiles], F32, tag="xp_part")
                for j in range(ntiles):
                    xt = xin_pool.tile([128, XROWS * W], F32, tag="xt")
                    xt3 = xt.rearrange("p (r c) -> p r c", c=W)
                    nc.sync.dma_start(
                        out=xt3, in_=x_d.ap()[n, :, j * XROWS : (j + 1) * XROWS, :]
                    )
                    xb = xbf_pool.tile([128, XROWS * W], BF16, tag="xb")
                    nc.vector.tensor_copy(xb[:, :], xt[:, :])
                    # maxpool partial over this 16-row band: out [128, 3]
                    xb4 = xb.rearrange("p (r kx c) -> p kx r c", kx=3, c=64)
                    nc.vector.tensor_reduce(
                        xp_part[:, 3 * j : 3 * j + 3],
                        xb4,
                        axis=mybir.AxisListType.XY,
                        op=mybir.AluOpType.max,
                    )
                    for r in range(XROWS // RROWS):
                        y0 = j * XROWS + r * RROWS
                        pf = psp.tile([128, 512], F32, tag="pf", name="pf")[
                            :, : RROWS * W
                        ]
                        nc.tensor.matmul(
                            pf[:, :],
                            w1t_sb[:, :],
                            xb[:, r * RROWS * W : (r + 1) * RROWS * W],
                        )
                        nc.scalar.activation(
                            fpad3[:, y0 + 1 : y0 + 1 + RROWS, 1 : 1 + W],
                            pf[c0 : c0 + CM, :].rearrange("p (r c) -> p r c", c=W),
                            mybir.ActivationFunctionType.Relu,
                            bias=b1n,
                        )

                # ---------------- finalize maxpool, compute g
                xp_r = sp.tile([128, 10], BF16, tag="xp_r")
                nc.vector.memset(xp_r[:, 9:10], 0.0)
                nc.vector.tensor_reduce(
                    xp_r[:, 0:9],
                    xp_part.rearrange(
                        "p (ky s kx) -> p ky kx s", ky=3, kx=3
                    ),
                    axis=mybir.AxisListType.X,
                    op=mybir.AluOpType.max,
                )
                pg = psp.tile([128, 512], F32, tag="pg", name="pg")[:, :10]
                nc.tensor.matmul(pg[:, :], w1t_sb[:, :], xp_r[:, :])
                nc.scalar.activation(
                    g2[c0 : c0 + CM, GCOL : GCOL + 9],
                    pg[c0 : c0 + CM, 0:9],
                    mybir.ActivationFunctionType.Relu,
                    bias=b1n,
                )

                # ---------------- MM2: T = G2.T @ f_pad, all padded rows
                tpl = mega[TPL : TPL + 10, :]
                for p0 in range(0, HP, RROWS):
                    pT = psp.tile([128, 512], F32, tag="pT", name="pT")[
                        :, : RROWS * WP
                    ]
                    nc.tensor.matmul(
                        pT[:, :],
                        g2[c0 : c0 + CM, :],
                        fpad[:, p0 * WP : (p0 + RROWS) * WP],
                    )
                    nc.vector.tensor_copy(
                        tpl[:, p0 * WP : (p0 + RROWS) * WP],
                        pT[TPL : TPL + 10, :],
                    )

                # ---------------- shifted tap copies SBUF->SBUF (HWDGE on sync)
                # Fixed even-sized chunks: odd-length bf16 copies lower to
                # 2-byte element-wise DMA descriptors (catastrophically slow).
                # Two 18818-elem chunks (37636B contiguous runs) cover every
                # tap range; the middle overlap rewrites identical data.
                CH = 18818
                for t in range(18):
                    src = TPL + (t if t < 9 else 9)
                    d = DELTAS[t % 9]
                    a = max(0, -d)
                    b = L - max(0, d)
                    for lo in (a, b - CH):
                        nc.sync.dma_start(
                            out=mega[TSB + t : TSB + t + 1, lo : lo + CH],
                            in_=mega[src : src + 1, lo + d : lo + CH + d],
                        )

                # ---------------- MM3 + bias + store
                tsb = mega[TSB : TSB + 18, :].rearrange("p (r c) -> p r c", c=WP)
                for j in range(ntiles):
                    ot = out_pool.tile([128, XROWS * W], F32, tag="ot")
                    for r in range(XROWS // RROWS):
                        y0 = j * XROWS + r * RROWS
                        po = psp.tile([128, 512], F32, tag="po", name="po")[
                            :, : RROWS * W
                        ]
                        nc.tensor.matmul(
                            po[:, :],
                            l3_sb[64:82, :],
                            tsb[:, y0 + 1 : y0 + 1 + RROWS, 1 : 1 + W],
                        )
                        nc.scalar.activation(
                            ot[:, r * RROWS * W : (r + 1) * RROWS * W],
                            po[:, :],
                            mybir.ActivationFunctionType.Identity,
                            bias=wb_sb[:, :],
                        )
                    nc.sync.dma_start(
                        out=out_d.ap()[n, :, j * XROWS : (j + 1) * XROWS, :],
                        in_=ot.rearrange("p (r c) -> p r c", c=W),
                    )
    return nc


_CACHE = {}


def _get_nc():
    if "nc" not in _CACHE:
        nc = bacc.Bacc(
            "TRN2", target_bir_lowering=False, debug=False, num_devices=N_CORES
        )
        build(nc)
        nc.compile()
        _CACHE["nc"] = nc
    return _CACHE["nc"]


def make_in_maps(x, W1, b1, Wk, bk, b_adap, Wf, bf):
    x = np.asarray(x, dtype=np.float32)
    W1 = np.asarray(W1, dtype=np.float32)
    b1 = np.asarray(b1, dtype=np.float32)
    Wk = np.asarray(Wk, dtype=np.float32)
    bk = np.asarray(bk, dtype=np.float32)
    b_adap = np.asarray(b_adap, dtype=np.float32)
    Wf = np.asarray(Wf, dtype=np.float32)
    bf = np.asarray(bf, dtype=np.float32)

    u = Wf @ Wk                # [128]
    v = Wf @ bk                # [128]
    w = Wf @ b_adap + bf       # [128]
    l3 = np.ascontiguousarray(
        np.stack([u] * 9 + [v] * 9).astype(ml_dtypes.bfloat16)
    )
    w1t = np.ascontiguousarray(
        np.tile(W1.T, (1, 4)).astype(ml_dtypes.bfloat16)
    )
    b1c = np.ascontiguousarray(b1[:, None].astype(np.float32))
    wbc = np.ascontiguousarray(w[:, None].astype(np.float32))

    in_maps = []
    for i in range(N_CORES):
        in_maps.append(
            {
                "x": np.ascontiguousarray(x[i * NS : (i + 1) * NS]),
                "w1t": w1t,
                "b1": b1c,
                "l3": l3,
                "wb": wbc,
            }
        )
    return in_maps


def kernel(x, W1, b1, Wk, bk, b_adap, Wf, bf):
    nc = _get_nc()
    in_maps = make_in_maps(x, W1, b1, Wk, bk, b_adap, Wf, bf)
    res = run_bass_kernel_spmd(nc, in_maps, list(range(N_CORES)))
    return np.concatenate([res.results[i]["out"] for i in range(N_CORES)], axis=0)


# revision 15
# speedup vs baseline: 9.4575x; 1.1788x over previous
"""Trainium2 Bass kernel for nn_AdaptiveKernelModule (dense_cnn).

Math: the per-sample dynamic conv kernel is rank-2 in its output channel:
    gk[o,i,kh,kw] = Wk[o] * g[i,kh,kw] + bk[o]
so with u = Wf@Wk, v = Wf@bk, w = Wf@b_adap + bf (host-precomputed):
    out[c, p] = u[c] * A[p] + v[c] * B[p] + w[c]
    A[p] = sum_{i,kh,kw} g[i,kh,kw] * f[i, p + delta(kh,kw)]
    B[p] = sum_{i,kh,kw}              f[i, p + delta(kh,kw)]
    f    = relu(W1 @ x + b1)

Device pipeline per sample (2 samples per core, 8 cores data-parallel over N):
  x arrives bf16 (host-converted; MM1 consumes bf16 anyway so no extra error)
  MM1: 4 chunks of f_psum = W1T.T @ x_chunk (bf16, K=128, M=32) packed into
       one [128,384] PSUM via tile_position col groups; ONE ACT relu+b1 evac
       writes the row-interleaved f_pad (partition group g = (row%8)//2)
  maxpool 64x64 windows on DVE -> xp; tiny MM (replicated W1T, M=128) + relu
       -> g taps written to all 4 row-groups of G2 in one ACT op
  MM2: T_psum = G2[32g].T @ f_pad block (bf16, K=32, M=128; G2 cols 96..104 =
       g taps, col 105 = ones), DVE copy rows 96..105 -> T_plain (flat padded)
  DMA SBUF->SBUF: T_sb[t, q] = T_plain[row(t), q + delta_t] (18 shifted rows,
       2 fixed 18818-elem chunks each; odd-length bf16 copies are pathological)
  MM3: out_psum = L3.T @ T_sb_chunk (bf16, K=18, M=128), two chunks per PSUM
       pair, ONE ACT Identity+bias(w) evac -> f32 out tile -> DMA to HBM.
"""

import numpy as np
import ml_dtypes

import concourse.bass as bass
import concourse.bacc as bacc
import concourse.mybir as mybir
import concourse.tile as tile
from concourse.bass_utils import run_bass_kernel_spmd

F32 = mybir.dt.float32
BF16 = mybir.dt.bfloat16

N_CORES = 8
NS = 2            # samples per core
C = 128           # input channels
CM = 32           # bottleneck channels
H = W = 192
HP = WP = 194     # padded
L = HP * WP       # padded pixels per plane (37636)
XROWS = 16        # image rows per x/out tile
RROWS = 2         # image rows per matmul chunk (N = 2*192 = 384)
NB = H // 8       # 24 8-row blocks per sample (f_pad interleaved layout)

# partition layout inside the T mega (bf16) tile
TSB = 64              # T_sb: 18 shifted tap rows
TPL = 96              # T_plain: 10 raw rows (9 g-taps + 1 ones-tap)
GCOL = 96             # G2 tap columns 96..104, ones col 105

DELTAS = [(kh - 1) * WP + (kw - 1) for kh in range(3) for kw in range(3)]


def build(nc):
    x_d = nc.declare_dram_parameter("x", [NS, C, H, W], BF16, isOutput=False)
    w1t4_d = nc.declare_dram_parameter("w1t4", [C, C], BF16, isOutput=False)
    b14_d = nc.declare_dram_parameter("b14", [C, 1], F32, isOutput=False)
    l3_d = nc.declare_dram_parameter("l3", [18, C], BF16, isOutput=False)
    wb_d = nc.declare_dram_parameter("wb", [C, 1], F32, isOutput=False)
    out_d = nc.declare_dram_parameter("out", [NS, C, H, W], F32, isOutput=True)

    with tile.TileContext(nc) as tc:
        with (
            tc.tile_pool(name="persist", bufs=1) as pp,
            tc.tile_pool(name="xbf", bufs=3) as xbf_pool,
            tc.tile_pool(name="outp", bufs=2) as out_pool,
            tc.tile_pool(name="small", bufs=2) as sp,
            tc.tile_pool(name="psum", bufs=2, space="PSUM") as psp,
        ):
            # T_plain (partitions 96..105) and T_sb (64..81), flat padded pixels
            tmeg = pp.tile([128, L], BF16)
            # f_pad, row-interleaved: f4[32*g + i, b*2*WP + r*WP + px] holds
            # f at padded row py = 8b + 2g + r + 1, padded col px (border cols
            # zero; border rows py=0/193 live only in T_plain as zeros)
            LF = NB * 2 * WP  # 24 blocks x 2 rows x 194 cols
            f4a = pp.tile([128, LF], BF16)
            f4b = pp.tile([128, LF], BF16)
            w1t4_sb = pp.tile([C, C], BF16)     # W1T replicated x4 (for MM-g)
            b14_sb = pp.tile([C, 1], F32)       # b1 tiled x4
            l3_sb = pp.tile([128, C], BF16)
            wb_sb = pp.tile([C, 1], F32)
            g2 = pp.tile([128, C], BF16)

            nc.sync.dma_start(out=w1t4_sb[:, :], in_=w1t4_d.ap())
            nc.sync.dma_start(out=b14_sb[:, :], in_=b14_d.ap())
            nc.sync.dma_start(out=l3_sb[64:82, :], in_=l3_d.ap())
            nc.sync.dma_start(out=wb_sb[:, :], in_=wb_d.ap())

            # G2: zero everywhere, ones in col GCOL+9 (all 4 row-groups);
            # per-sample g taps land in cols GCOL..GCOL+8 via one ACT evac.
            nc.vector.memset(g2[:, :], 0.0)
            nc.vector.memset(g2[:, GCOL + 9 : GCOL + 10], 1.0)

            # T_plain border rows stay zero forever (MM2 covers py 1..192)
            nc.vector.memset(tmeg[TPL : TPL + 10, 0:WP], 0.0)
            nc.vector.memset(tmeg[TPL : TPL + 10, 193 * WP : 194 * WP], 0.0)

            # f_pad border columns (px = 0 and 193) stay zero forever
            for f4 in (f4a, f4b):
                f4v = f4.rearrange("p (b r c) -> p b r c", r=2, c=WP)
                nc.vector.memset(f4v[:, :, :, 0:1], 0.0)
                nc.vector.memset(f4v[:, :, :, WP - 1 : WP], 0.0)

            for n in range(NS):
                f4 = f4a if n % 2 == 0 else f4b

                # -------- pass 1: x in (bf16), maxpool partials, MM1+relu
                ntiles = H // XROWS  # 12
                xp_part = sp.tile([128, 3 * ntiles], F32, tag="xp_part")
                for j in range(ntiles):
                    xb = xbf_pool.tile([128, XROWS * W], BF16, tag="xb")
                    xb3 = xb.rearrange("p (r c) -> p r c", c=W)
                    nc.sync.dma_start(
                        out=xb3, in_=x_d.ap()[n, :, j * XROWS : (j + 1) * XROWS, :]
                    )
                    xb4 = xb.rearrange("p (r kx c) -> p kx r c", kx=3, c=64)
                    nc.vector.tensor_reduce(
                        xp_part[:, 3 * j : 3 * j + 3],
                        xb4,
                        axis=mybir.AxisListType.XY,
                        op=mybir.AluOpType.max,
                    )
                    for half in range(2):
                        J = 2 * j + half  # 8-row block index b
                        pf = psp.tile([128, 512], F32, tag="pf", name="pf")[
                            :, : RROWS * W
                        ]
                        for g in range(4):
                            yloc = half * 8 + 2 * g  # row offset within x tile
                            nc.tensor.matmul(
                                pf[32 * g : 32 * g + 32, :],
                                w1t4_sb[:, 32 * g : 32 * g + 32],
                                xb[:, yloc * W : (yloc + 2) * W],
                                tile_position=(0, 32 * g),
                            )
                        # one [128, 2, 192] relu evac -> interleaved f_pad
                        nc.scalar.activation(
                            f4.rearrange("p (b r c) -> p b r c", r=2, c=WP)[
                                :, J, :, 1 : 1 + W
                            ],
                            pf.rearrange("p (r c) -> p r c", c=W),
                            mybir.ActivationFunctionType.Relu,
                            bias=b14_sb[:, :],
                        )

                # -------- finalize maxpool, compute g (all 4 G2 row-groups)
                xp_r = sp.tile([128, 10], BF16, tag="xp_r")
                nc.vector.memset(xp_r[:, 9:10], 0.0)
                nc.vector.tensor_reduce(
                    xp_r[:, 0:9],
                    xp_part.rearrange("p (ky s kx) -> p ky kx s", ky=3, kx=3),
                    axis=mybir.AxisListType.X,
                    op=mybir.AluOpType.max,
                )
                pg = psp.tile([128, 512], F32, tag="pT", name="pg")[:, :10]
                nc.tensor.matmul(pg[:, :], w1t4_sb[:, :], xp_r[:, :])
                nc.scalar.activation(
                    g2[:, GCOL : GCOL + 9],
                    pg[:, 0:9],
                    mybir.ActivationFunctionType.Relu,
                    bias=b14_sb[:, :],
                )

                # -------- MM2: T rows for padded rows 1..192, block by block
                tpl = tmeg[TPL : TPL + 10, :]
                for b in range(NB):
                    for g in range(4):
                        py = 8 * b + 2 * g + 1
                        pT = psp.tile([128, 512], F32, tag="pT", name="pT")[
                            :, : RROWS * WP
                        ]
                        nc.tensor.matmul(
                            pT[:, :],
                            g2[32 * g : 32 * g + 32, :],
                            f4.rearrange("p (b f) -> p b f", f=2 * WP)[
                                32 * g : 32 * g + 32, b, :
                            ],
                            tile_position=(32 * g, 0),
                        )
                        nc.vector.tensor_copy(
                            tpl[:, py * WP : (py + 2) * WP],
                            pT[TPL : TPL + 10, :],
                        )

                # -------- shifted tap copies SBUF->SBUF (HWDGE on sync)
                CH = 18818
                for t in range(18):
                    src = TPL + (t if t < 9 else 9)
                    d = DELTAS[t % 9]
                    a = max(0, -d)
                    bb = L - max(0, d)
                    for lo in (a, bb - CH):
                        nc.sync.dma_start(
                            out=tmeg[TSB + t : TSB + t + 1, lo : lo + CH],
                            in_=tmeg[src : src + 1, lo + d : lo + CH + d],
                        )

                # -------- MM3 + bias + store (2 chunks per PSUM pair)
                tsb = tmeg[TSB : TSB + 18, :].rearrange("p (r c) -> p r c", c=WP)
                for j in range(ntiles):
                    ot = out_pool.tile([128, XROWS * W], F32, tag="ot")
                    for half in range(4):
                        po = psp.tile([128, 1024], F32, tag="po", name="po")
                        for q in range(2):
                            y0 = j * XROWS + (half * 2 + q) * RROWS
                            nc.tensor.matmul(
                                po[:, q * 512 : q * 512 + RROWS * W],
                                l3_sb[64:82, :],
                                tsb[:, y0 + 1 : y0 + 1 + RROWS, 1 : 1 + W],
                            )
                        nc.scalar.activation(
                            ot[:, half * 2 * RROWS * W : (half + 1) * 2 * RROWS * W],
                            po.rearrange("p (q f) -> p q f", q=2)[:, :, : RROWS * W],
                            mybir.ActivationFunctionType.Identity,
                            bias=wb_sb[:, :],
                        )
                    nc.sync.dma_start(
                        out=out_d.ap()[n, :, j * XROWS : (j + 1) * XROWS, :],
                        in_=ot.rearrange("p (r c) -> p r c", c=W),
                    )
    return nc


_CACHE = {}


def _get_nc():
    if "nc" not in _CACHE:
        nc = bacc.Bacc(
            "TRN2", target_bir_lowering=False, debug=False, num_devices=N_CORES
        )
        build(nc)
        nc.compile()
        _CACHE["nc"] = nc
    return _CACHE["nc"]


def make_in_maps(x, W1, b1, Wk, bk, b_adap, Wf, bf):
    x = np.asarray(x, dtype=np.float32)
    W1 = np.asarray(W1, dtype=np.float32)
    b1 = np.asarray(b1, dtype=np.float32)
    Wk = np.asarray(Wk, dtype=np.float32)
    bk = np.asarray(bk, dtype=np.float32)
    b_adap = np.asarray(b_adap, dtype=np.float32)
    Wf = np.asarray(Wf, dtype=np.float32)
    bf = np.asarray(bf, dtype=np.float32)

    u = Wf @ Wk                # [128]
    v = Wf @ bk                # [128]
    w = Wf @ b_adap + bf       # [128]
    l3 = np.ascontiguousarray(np.stack([u] * 9 + [v] * 9).astype(ml_dtypes.bfloat16))
    w1t4 = np.ascontiguousarray(np.tile(W1.T, (1, 4)).astype(ml_dtypes.bfloat16))
    b14 = np.ascontiguousarray(np.tile(b1, 4)[:, None].astype(np.float32))
    wbc = np.ascontiguousarray(w[:, None].astype(np.float32))
    xb = np.ascontiguousarray(x.astype(ml_dtypes.bfloat16))

    in_maps = []
    for i in range(N_CORES):
        in_maps.append(
            {
                "x": xb[i * NS : (i + 1) * NS],
                "w1t4": w1t4,
                "b14": b14,
                "l3": l3,
                "wb": wbc,
            }
        )
    return in_maps


def kernel(x, W1, b1, Wk, bk, b_adap, Wf, bf):
    nc = _get_nc()
    in_maps = make_in_maps(x, W1, b1, Wk, bk, b_adap, Wf, bf)
    res = run_bass_kernel_spmd(nc, in_maps, list(range(N_CORES)))
    return np.concatenate([res.results[i]["out"] for i in range(N_CORES)], axis=0)


# revision 16
# speedup vs baseline: 10.9407x; 1.1568x over previous
"""Trainium2 Bass kernel for nn_AdaptiveKernelModule (dense_cnn).

Math: the per-sample dynamic conv kernel is rank-2 in its output channel:
    gk[o,i,kh,kw] = Wk[o] * g[i,kh,kw] + bk[o]
so with u = Wf@Wk, v = Wf@bk, w = Wf@b_adap + bf (host-precomputed):
    out[c, p] = u[c] * A[p] + v[c] * B[p] + w[c]
    A[p] = sum_{i,kh,kw} g[i,kh,kw] * f[i, p + delta(kh,kw)]
    B[p] = sum_{i,kh,kw}              f[i, p + delta(kh,kw)]
    f    = relu(W1 @ x + b1)

Device pipeline per sample (2 samples per core, 8 cores data-parallel over N):
  x arrives bf16 (host-converted; MM1 consumes bf16 anyway, identical result)
  MM1: 8 chunks of f_psum = W1T.T @ x_chunk (bf16, K=128, M=32) packed into
       one [128, 2x512] PSUM pair via col groups; ONE ACT relu+b1 evac per
       x-tile writes the row-interleaved f_pad (partition group g=(row%8)//2)
  maxpool 64x64 windows on DVE -> xp; tiny MM (replicated W1T, M=128) + relu
       -> g taps written to all 4 row-groups of G2 in one ACT op
  MM2: T_psum = G2[32g].T @ f_pad block (bf16, K=32, M=10 at psum rows
       96..105), two chunks per PSUM pair, ONE DVE copy -> T_plain (flat)
  DMA SBUF->SBUF: T_sb[t, q] = T_plain[row(t), q + delta_t] (18 shifted rows,
       2 fixed 18818-elem chunks each; odd-length bf16 copies are pathological)
  MM3: out_psum = L3.T @ T_sb_chunk (bf16, K=18, M=128), two chunks per PSUM
       pair, ONE ACT Identity+bias(w) evac -> f32 out tile -> DMA to HBM.
All PSUM traffic shares one 4-slot x 2-bank pool so phases time-share the 8
banks and stay double-buffered.
"""

import numpy as np
import ml_dtypes

import concourse.bass as bass
import concourse.bacc as bacc
import concourse.mybir as mybir
import concourse.tile as tile
from concourse.bass_utils import run_bass_kernel_spmd

F32 = mybir.dt.float32
BF16 = mybir.dt.bfloat16

N_CORES = 8
NS = 2            # samples per core
C = 128           # input channels
CM = 32           # bottleneck channels
H = W = 192
HP = WP = 194     # padded
L = HP * WP       # padded pixels per plane (37636)
XROWS = 16        # image rows per x/out tile
RROWS = 2         # image rows per matmul chunk (N = 2*192 = 384)
NB = H // 8       # 24 8-row blocks per sample (f_pad interleaved layout)

TSB = 64              # T_sb partitions: 18 shifted tap rows
TPL = 96              # T_plain partitions: 10 raw rows (9 g-taps + ones-tap)
GCOL = 96             # G2 tap columns 96..104, ones col 105

DELTAS = [(kh - 1) * WP + (kw - 1) for kh in range(3) for kw in range(3)]


def build(nc):
    x_d = nc.declare_dram_parameter("x", [NS, C, H, W], BF16, isOutput=False)
    w1t4_d = nc.declare_dram_parameter("w1t4", [C, C], BF16, isOutput=False)
    b14_d = nc.declare_dram_parameter("b14", [C, 1], F32, isOutput=False)
    l3_d = nc.declare_dram_parameter("l3", [18, C], BF16, isOutput=False)
    wb_d = nc.declare_dram_parameter("wb", [C, 1], F32, isOutput=False)
    out_d = nc.declare_dram_parameter("out", [NS, C, H, W], F32, isOutput=True)

    with tile.TileContext(nc) as tc:
        with (
            tc.tile_pool(name="persist", bufs=1) as pp,
            tc.tile_pool(name="xbf", bufs=3) as xbf_pool,
            tc.tile_pool(name="outp", bufs=2) as out_pool,
            tc.tile_pool(name="small", bufs=2) as sp,
            tc.tile_pool(name="psum", bufs=4, space="PSUM") as psp,
        ):
            # T_plain (partitions 96..105) and T_sb (64..81), flat padded pixels
            tmeg = pp.tile([128, L], BF16)
            # f_pad, row-interleaved: f4[32*g + i, b*2*WP + r*WP + px] holds
            # f at padded row py = 8b + 2g + r + 1, padded col px
            LF = NB * 2 * WP
            f4a = pp.tile([128, LF], BF16)
            f4b = pp.tile([128, LF], BF16)
            w1t4_sb = pp.tile([C, C], BF16)     # W1T replicated x4
            b14_sb = pp.tile([C, 1], F32)       # b1 tiled x4
            l3_sb = pp.tile([128, C], BF16)
            wb_sb = pp.tile([C, 1], F32)
            g2 = pp.tile([128, C], BF16)

            nc.sync.dma_start(out=w1t4_sb[:, :], in_=w1t4_d.ap())
            nc.sync.dma_start(out=b14_sb[:, :], in_=b14_d.ap())
            nc.sync.dma_start(out=l3_sb[64:82, :], in_=l3_d.ap())
            nc.sync.dma_start(out=wb_sb[:, :], in_=wb_d.ap())

            nc.vector.memset(g2[:, :], 0.0)
            nc.vector.memset(g2[:, GCOL + 9 : GCOL + 10], 1.0)

            # T_plain border rows stay zero forever (MM2 covers py 1..192)
            nc.vector.memset(tmeg[TPL : TPL + 10, 0:WP], 0.0)
            nc.vector.memset(tmeg[TPL : TPL + 10, 193 * WP : 194 * WP], 0.0)

            # f_pad border columns (px = 0 and 193) stay zero forever
            for f4 in (f4a, f4b):
                f4v = f4.rearrange("p (b r c) -> p b r c", r=2, c=WP)
                nc.vector.memset(f4v[:, :, :, 0:1], 0.0)
                nc.vector.memset(f4v[:, :, :, WP - 1 : WP], 0.0)

            for n in range(NS):
                f4 = f4a if n % 2 == 0 else f4b

                # -------- pass 1: x in (bf16), maxpool partials, MM1+relu
                ntiles = H // XROWS  # 12
                xp_part = sp.tile([128, 3 * ntiles], F32, tag="xp_part")
                for j in range(ntiles):
                    xb = xbf_pool.tile([128, XROWS * W], BF16, tag="xb")
                    xb3 = xb.rearrange("p (r c) -> p r c", c=W)
                    nc.sync.dma_start(
                        out=xb3, in_=x_d.ap()[n, :, j * XROWS : (j + 1) * XROWS, :]
                    )
                    xb4 = xb.rearrange("p (r kx c) -> p kx r c", kx=3, c=64)
                    nc.vector.tensor_reduce(
                        xp_part[:, 3 * j : 3 * j + 3],
                        xb4,
                        axis=mybir.AxisListType.XY,
                        op=mybir.AluOpType.max,
                    )
                    ps = psp.tile([128, 1024], F32, tag="ps", name="psf")
                    for half in range(2):
                        for g in range(4):
                            yloc = half * 8 + 2 * g
                            nc.tensor.matmul(
                                ps[32 * g : 32 * g + 32, half * 512 : half * 512 + RROWS * W],
                                w1t4_sb[:, 32 * g : 32 * g + 32],
                                xb[:, yloc * W : (yloc + 2) * W],
                                tile_position=(0, 32 * g),
                            )
                    # one [128, 2, 2, 192] relu evac -> interleaved f_pad
                    nc.scalar.activation(
                        f4.rearrange("p (b r c) -> p b r c", r=2, c=WP)[
                            :, 2 * j : 2 * j + 2, :, 1 : 1 + W
                        ],
                        ps.rearrange("p (h f) -> p h f", h=2)[:, :, : RROWS * W]
                        .rearrange("p h (r c) -> p h r c", c=W),
                        mybir.ActivationFunctionType.Relu,
                        bias=b14_sb[:, :],
                    )

                # -------- finalize maxpool, compute g (all 4 G2 row-groups)
                xp_r = sp.tile([128, 10], BF16, tag="xp_r")
                nc.vector.memset(xp_r[:, 9:10], 0.0)
                nc.vector.tensor_reduce(
                    xp_r[:, 0:9],
                    xp_part.rearrange("p (ky s kx) -> p ky kx s", ky=3, kx=3),
                    axis=mybir.AxisListType.X,
                    op=mybir.AluOpType.max,
                )
                pg = psp.tile([128, 1024], F32, tag="ps", name="pg")[:, :10]
                nc.tensor.matmul(pg[:, :], w1t4_sb[:, :], xp_r[:, :])
                nc.scalar.activation(
                    g2[:, GCOL : GCOL + 9],
                    pg[:, 0:9],
                    mybir.ActivationFunctionType.Relu,
                    bias=b14_sb[:, :],
                )

                # -------- MM2: T rows for padded rows 1..192, in py order,
                # two chunks per PSUM pair, one DVE evac per pair
                tpl = tmeg[TPL : TPL + 10, :]
                f4r = f4.rearrange("p (b f) -> p b f", f=2 * WP)
                for b in range(NB):
                    for gp in range(2):  # g pairs: (0,1) and (2,3)
                        pT = psp.tile([128, 1024], F32, tag="ps", name="pT")
                        for q in range(2):
                            g = 2 * gp + q
                            nc.tensor.matmul(
                                pT[TPL : TPL + 10, q * 512 : q * 512 + 2 * WP],
                                g2[32 * g : 32 * g + 32, GCOL : GCOL + 10],
                                f4r[32 * g : 32 * g + 32, b, :],
                                tile_position=(32 * g, TPL),
                            )
                        py0 = 8 * b + 4 * gp + 1
                        nc.vector.tensor_copy(
                            tpl[:, py0 * WP : (py0 + 4) * WP],
                            pT.rearrange("p (q f) -> p q f", q=2)[
                                TPL : TPL + 10, :, : 2 * WP
                            ],
                        )

                # -------- shifted tap copies SBUF->SBUF (HWDGE on sync)
                CH = 18818
                for t in range(18):
                    src = TPL + (t if t < 9 else 9)
                    d = DELTAS[t % 9]
                    a = max(0, -d)
                    bb = L - max(0, d)
                    for lo in (a, bb - CH):
                        nc.sync.dma_start(
                            out=tmeg[TSB + t : TSB + t + 1, lo : lo + CH],
                            in_=tmeg[src : src + 1, lo + d : lo + CH + d],
                        )

                # -------- MM3 + bias + store (2 chunks per PSUM pair)
                tsb = tmeg[TSB : TSB + 18, :].rearrange("p (r c) -> p r c", c=WP)
                for j in range(ntiles):
                    ot = out_pool.tile([128, XROWS * W], F32, tag="ot")
                    for half in range(4):
                        po = psp.tile([128, 1024], F32, tag="ps", name="po")
                        for q in range(2):
                            y0 = j * XROWS + (half * 2 + q) * RROWS
                            nc.tensor.matmul(
                                po[:, q * 512 : q * 512 + RROWS * W],
                                l3_sb[64:82, :],
                                tsb[:, y0 + 1 : y0 + 1 + RROWS, 1 : 1 + W],
                            )
                        nc.scalar.activation(
                            ot[:, half * 2 * RROWS * W : (half + 1) * 2 * RROWS * W],
                            po.rearrange("p (q f) -> p q f", q=2)[:, :, : RROWS * W],
                            mybir.ActivationFunctionType.Identity,
                            bias=wb_sb[:, :],
                        )
                    nc.sync.dma_start(
                        out=out_d.ap()[n, :, j * XROWS : (j + 1) * XROWS, :],
                        in_=ot.rearrange("p (r c) -> p r c", c=W),
                    )
    return nc


_CACHE = {}


def _get_nc():
    if "nc" not in _CACHE:
        nc = bacc.Bacc(
            "TRN2", target_bir_lowering=False, debug=False, num_devices=N_CORES
        )
        build(nc)
        nc.compile()
        _CACHE["nc"] = nc
    return _CACHE["nc"]


def make_in_maps(x, W1, b1, Wk, bk, b_adap, Wf, bf):
    x = np.asarray(x, dtype=np.float32)
    W1 = np.asarray(W1, dtype=np.float32)
    b1 = np.asarray(b1, dtype=np.float32)
    Wk = np.asarray(Wk, dtype=np.float32)
    bk = np.asarray(bk, dtype=np.float32)
    b_adap = np.asarray(b_adap, dtype=np.float32)
    Wf = np.asarray(Wf, dtype=np.float32)
    bf = np.asarray(bf, dtype=np.float32)

    u = Wf @ Wk                # [128]
    v = Wf @ bk                # [128]
    w = Wf @ b_adap + bf       # [128]
    l3 = np.ascontiguousarray(np.stack([u] * 9 + [v] * 9).astype(ml_dtypes.bfloat16))
    w1t4 = np.ascontiguousarray(np.tile(W1.T, (1, 4)).astype(ml_dtypes.bfloat16))
    b14 = np.ascontiguousarray(np.tile(b1, 4)[:, None].astype(np.float32))
    wbc = np.ascontiguousarray(w[:, None].astype(np.float32))
    xb = np.ascontiguousarray(x.astype(ml_dtypes.bfloat16))

    in_maps = []
    for i in range(N_CORES):
        in_maps.append(
            {
                "x": xb[i * NS : (i + 1) * NS],
                "w1t4": w1t4,
                "b14": b14,
                "l3": l3,
                "wb": wbc,
            }
        )
    return in_maps


def kernel(x, W1, b1, Wk, bk, b_adap, Wf, bf):
    nc = _get_nc()
    in_maps = make_in_maps(x, W1, b1, Wk, bk, b_adap, Wf, bf)
    res = run_bass_kernel_spmd(nc, in_maps, list(range(N_CORES)))
    return np.concatenate([res.results[i]["out"] for i in range(N_CORES)], axis=0)
